# revision 12
# baseline (speedup 1.0000x reference)
"""Fused multi-head attention for Trainium2 (Bass/Tile), 8-core SPMD.

Problem: B=2, H=16, S=4096, D=64, fp32, mask == all-ones (unmasked softmax).

Strategy (per core, 4 of the 32 (b,h) heads):
  * S^T orientation flash attention: keys on partitions, queries on the free
    dim, so no on-chip transposes are needed anywhere.
  * QK^T: lhsT = K^T tile [64, 128] (fp32r), rhs = Q^T block [64, 512]
    (fp32r) -> S^T psum tile [128 keys, 512 queries]. K=64 contraction runs
    in the PE's 64-row tiling mode; even key-tiles use array rows 0-63, odd
    key-tiles rows 64-127, so pairs execute concurrently.
  * exp on ScalarE straight out of PSUM in 1536-wide chunks (scale=1/8
    folded into the activation), output bf16 P^T tiles to SBUF.
  * P@V: V is pre-augmented host-side with a ones column (V' = [V, 1]) so
    the 65th output row accumulates the softmax denominator for free.
    Each 128-key tile is split into two 64-key halves on rows 0-63/64-127
    (again concurrent 64-row-mode pairs) accumulating into two PSUM banks;
    a single DVE add merges them.
  * Normalization (divide by denominator) and the final [D, S] -> [S, D]
    transpose happen host-side on the gathered outputs.

Inputs are pre-rearranged host-side (numpy) into the layouts the kernel
wants: Q^T duplicated onto both partition halves, K^T even/odd-packed, and
V' key-tile-major in bf16.
"""

import numpy as np

import concourse.bass as bass
import concourse.mybir as mybir
import concourse.tile as tile
from concourse import bacc
from concourse.bass_utils import run_bass_kernel_spmd

B, H, S, D = 2, 16, 4096, 64
BH = B * H
N_CORES = 8
NH = BH // N_CORES          # heads per core
QB = 512                    # queries per q-block
N_QB = S // QB              # q-blocks per head
KT = S // 128               # 128-key tiles per head
CHUNK = 3                   # key-tiles per exp chunk (3 psum banks)

F32 = mybir.dt.float32
F32R = mybir.dt.float32r
BF16 = mybir.dt.bfloat16

_cache = {}


def _build_program():
    nc = bacc.Bacc()
    kt_in = nc.declare_dram_parameter("kt", [NH, 128, S // 2], F32R, isOutput=False)
    qt_in = nc.declare_dram_parameter("qt", [NH, 128, S], F32R, isOutput=False)
    v_in = nc.declare_dram_parameter("v", [NH, 128, KT * 65], F32R, isOutput=False)
    o_out = nc.declare_dram_parameter("o", [NH, 65, S], F32, isOutput=True)

    with tile.TileContext(nc) as tc:
        with (
            tc.tile_pool(name="kt_p", bufs=2) as kt_pool,
            tc.tile_pool(name="qt_p", bufs=2) as qt_pool,
            tc.tile_pool(name="v_p", bufs=2) as v_pool,
            tc.tile_pool(name="pt_p", bufs=8) as pt_pool,
            tc.tile_pool(name="osum_p", bufs=2) as osum_pool,
            tc.tile_pool(name="stage_p", bufs=2, space="PSUM") as stage_pool,
            tc.tile_pool(name="ot_p", bufs=2, space="PSUM") as ot_pool,
        ):
            class PVState:
                """Previous q-block's P@V, emitted chunk-by-chunk between
                the exp chunks so the PE never bursts long enough to starve
                ScalarE. P^T arrives as per-chunk fp32 tiles."""

                def __init__(self, v_s, h, qb):
                    self.v_s, self.h, self.qb = v_s, h, qb
                    self.k = 0
                    self.queue = []
                    self.ot_a = ot_pool.tile([128, QB], F32, tag="ot")
                    self.ot_b = ot_pool.tile([128, QB], F32, tag="ot")

                def add_chunk(self, pt, csz):
                    self.queue.append((pt, csz))

                def emit_chunk(self):
                    pt, csz = self.queue.pop(0)
                    for i in range(csz):
                        k = self.k + i
                        for half, ot in ((0, self.ot_a), (1, self.ot_b)):
                            lhsT = self.v_s[64 * half:64 * half + 64,
                                            k * 65:(k + 1) * 65]
                            rhs = pt[64 * half:64 * half + 64,
                                     i * QB:(i + 1) * QB]
                            nc.tensor.matmul(
                                ot[0:65, :], lhsT, rhs,
                                start=(k == 0), stop=(k == KT - 1),
                                skip_group_check=True,
                            )
                    self.k += csz

                def finish(self):
                    while self.queue:
                        self.emit_chunk()
                    assert self.k == KT
                    osum = osum_pool.tile([128, QB], F32, tag="osum")
                    nc.vector.tensor_copy(osum[0:65, :], self.ot_a[0:65, :])
                    nc.vector.tensor_add(
                        osum[0:65, :], osum[0:65, :], self.ot_b[0:65, :]
                    )
                    nc.sync.dma_start(
                        o_out[self.h, :, self.qb * QB:(self.qb + 1) * QB],
                        osum[0:65, :],
                    )

            def chunked_load(dst, src, widths):
                c0 = 0
                for w in widths:
                    nc.gpsimd.dma_start(dst[:, c0:c0 + w], src[:, c0:c0 + w])
                    c0 += w
                assert c0 == dst.shape[-1]

            chunk_sizes = [CHUNK] * (KT // CHUNK) + (
                [KT % CHUNK] if KT % CHUNK else []
            )
            n_chunks = len(chunk_sizes)

            prev = None    # PV of previous q-block: last 2 chunks + flush left
            cur = None     # PV of current q-block, trailing the exp by 2 chunks
            for h in range(NH):
                # Need-order for the first head: every q-block scans all of
                # K^T, so K^T fully gates q-block 0; then Q^T's first block,
                # then V (first needed by the q-block-0 P@V one block later).
                kt_s = kt_pool.tile([128, S // 2], F32R, tag="kt")
                qt_s = qt_pool.tile([128, S], F32R, tag="qt")
                v_s = v_pool.tile([128, KT * 65], F32R, tag="v")
                for j in range(8):
                    w = S // 16
                    nc.gpsimd.dma_start(
                        kt_s[:, j * w:(j + 1) * w], kt_in[h][:, j * w:(j + 1) * w]
                    )
                nc.gpsimd.dma_start(qt_s[:, 0:QB], qt_in[h][:, 0:QB])
                chunked_load(
                    v_s[:, :], v_in[h][:, :], [520, 520, 520, 520]
                )
                c0 = QB
                for w in [QB] * 7:
                    nc.gpsimd.dma_start(
                        qt_s[:, c0:c0 + w], qt_in[h][:, c0:c0 + w]
                    )
                    c0 += w

                for qb in range(N_QB):
                    cur = PVState(v_s, h, qb)
                    col = 0
                    for c, csz in enumerate(chunk_sizes):
                        st = stage_pool.tile([128, csz * QB], F32, tag="stage")
                        for i in range(csz):
                            k = col + i
                            half = k % 2
                            blk = k // 2
                            lhsT = kt_s[64 * half:64 * half + 64,
                                        blk * 128:(blk + 1) * 128]
                            rhs = qt_s[64 * half:64 * half + 64,
                                       qb * QB:(qb + 1) * QB]
                            nc.tensor.matmul(
                                st[:, i * QB:(i + 1) * QB], lhsT, rhs,
                                start=True, stop=True,
                            )
                        pt = pt_pool.tile([128, csz * QB], F32R, tag="pt")
                        nc.scalar.activation(
                            pt[:, :],
                            st[:, :csz * QB],
                            mybir.ActivationFunctionType.Exp,
                            scale=1.0 / np.sqrt(float(D)),
                        )
                        cur.add_chunk(pt, csz)
                        col += csz
                        # PE filler between exp chunks: drain the previous
                        # q-block's PV leftovers first, then this q-block's
                        # PV trailing two chunks behind the exp chain.
                        if c == 0:
                            if prev is not None:
                                prev.emit_chunk()
                        elif c == 1:
                            if prev is not None:
                                prev.finish()
                                prev = None
                        else:
                            cur.emit_chunk()
                    prev = cur
            prev.finish()

    nc.compile()
    return nc


def _get_program():
    if "nc" not in _cache:
        _cache["nc"] = _build_program()
    return _cache["nc"]


def _pack_inputs(Q, K, V):
    """Host-side rearrangement into per-core device layouts."""
    Qf = np.ascontiguousarray(Q.reshape(BH, S, D))
    Kf = np.ascontiguousarray(K.reshape(BH, S, D))
    Vf = np.ascontiguousarray(V.reshape(BH, S, D))

    # Q^T [BH, 64, S], duplicated onto both partition halves -> [BH, 128, S]
    QT = Qf.transpose(0, 2, 1)
    QTd = np.ascontiguousarray(np.concatenate([QT, QT], axis=1), dtype=np.float32)

    # K^T [BH, 64, S] -> even key-tiles on partitions 0-63, odd on 64-127
    KTm = Kf.transpose(0, 2, 1).reshape(BH, D, KT, 128)
    KTpack = np.concatenate(
        [
            KTm[:, :, 0::2, :].reshape(BH, D, S // 2),
            KTm[:, :, 1::2, :].reshape(BH, D, S // 2),
        ],
        axis=1,
    ).astype(np.float32)

    # V' = [V, ones]; key-tile-major bf16 layout [BH, 128, KT*65]
    Vp = np.concatenate([Vf, np.ones((BH, S, 1), np.float32)], axis=-1)
    Vb = np.ascontiguousarray(
        Vp.reshape(BH, KT, 128, 65)
        .transpose(0, 2, 1, 3)
        .reshape(BH, 128, KT * 65)
    )
    return KTpack, QTd, Vb


def ml_bfloat16():
    import ml_dtypes

    return ml_dtypes.bfloat16


def kernel(Q, K, V, mask):
    assert Q.shape == (B, H, S, D)
    nc = _get_program()
    KTpack, QTd, Vb = _pack_inputs(
        np.asarray(Q, dtype=np.float32),
        np.asarray(K, dtype=np.float32),
        np.asarray(V, dtype=np.float32),
    )
    in_maps = []
    for c in range(N_CORES):
        sl = slice(c * NH, (c + 1) * NH)
        in_maps.append(
            {
                "kt": np.ascontiguousarray(KTpack[sl]),
                "qt": np.ascontiguousarray(QTd[sl]),
                "v": np.ascontiguousarray(Vb[sl]),
            }
        )
    res = run_bass_kernel_spmd(nc, in_maps, core_ids=list(range(N_CORES)))
    O = np.concatenate([r["o"] for r in res.results], axis=0)  # [BH, 65, S]
    out = (O[:, :D, :] / O[:, D:D + 1, :]).transpose(0, 2, 1)  # [BH, S, D]
    return np.ascontiguousarray(out.reshape(B, H, S, D).astype(np.float32))
